# revision 1
# baseline (speedup 1.0000x reference)
"""Trainium2 Bass kernel for nn_CrossAttention (b=2, n=m=2048, dim=1024, 16 heads x 64).

Sharding: 8 cores = (batch b in {0,1}) x (head-group g in {0..3}, 4 heads each).
Per core: project q/k/v for its 4 heads (feature-major layouts), attention with
softmax (no max subtraction -- logits are bounded ~|2.7|), row sums via a ones
column appended to V, then an AllToAll over the 4 cores of each batch converts
head-sharding to row-sharding for the output MLP (relu(A@W1)@W2) + LayerNorm.
"""

import sys

if "/opt/trn_rl_repo" not in sys.path:
    sys.path.insert(0, "/opt/trn_rl_repo")

from contextlib import ExitStack

import numpy as np
import ml_dtypes

import concourse.bacc as bacc
import concourse.tile as tile
from concourse import mybir, library_config
from concourse.bass_utils import run_bass_kernel_spmd

DT = mybir.dt
BF16 = ml_dtypes.bfloat16

P = 128          # partitions
N = 2048         # tokens per batch
DIM = 1024       # model dim
HD = 64          # head dim
NH = 4           # heads per core
E = NH * HD      # 256 features per core
CT = DIM // P    # 8 contraction tiles
JT = N // P      # 16 key tiles
IBS = 512        # i-block size
IB = N // IBS    # 4 i-blocks
RQ = 512         # output rows per core
FT = DIM // P    # 8 f-tiles in MLP

_PROGRAM = None
LAST_RUN = None  # BassKernelResults of the most recent kernel() call


def build_program():
    nc = bacc.Bacc(None, num_devices=8)

    xT_d = nc.dram_tensor("xT", [DIM, N], DT.bfloat16, kind="ExternalInput")
    ctxT_d = nc.dram_tensor("ctxT", [DIM, N], DT.bfloat16, kind="ExternalInput")
    wq_d = nc.dram_tensor("wq", [DIM, E], DT.bfloat16, kind="ExternalInput")
    wk_d = nc.dram_tensor("wk", [DIM, E], DT.bfloat16, kind="ExternalInput")
    wv_d = nc.dram_tensor("wv", [DIM, E], DT.bfloat16, kind="ExternalInput")
    w1_d = nc.dram_tensor("w1", [DIM, DIM], DT.bfloat16, kind="ExternalInput")
    w2_d = nc.dram_tensor("w2", [DIM, DIM], DT.bfloat16, kind="ExternalInput")
    gamma_d = nc.dram_tensor("gamma", [1, DIM], DT.float32, kind="ExternalInput")
    out_d = nc.dram_tensor("out", [RQ, DIM], DT.float32, kind="ExternalOutput")

    with tile.TileContext(nc) as tc:
        stack = ExitStack()
        with stack:
            nc.gpsimd.load_library(library_config.attnmlp)

            const = stack.enter_context(tc.tile_pool(name="const", bufs=1))
            gamma_sb = const.tile([1, DIM], DT.float32, name="gamma_sb", tag="gamma_sb")
            nc.sync.dma_start(gamma_sb[:], gamma_d[:])
            gamma_bc = const.tile([P, DIM], DT.float32, name="gamma_bc", tag="gamma_bc")
            nc.gpsimd.partition_broadcast(gamma_bc[:], gamma_sb[:])
            eps_t = const.tile([P, 1], DT.float32, name="eps_t", tag="eps_t")
            nc.vector.memset(eps_t[:], 1e-5)

            # ---- persistent activation tiles ----
            qkv = stack.enter_context(tc.tile_pool(name="qkv", bufs=1))
            qT_t = [qkv.tile([P, N], DT.bfloat16, name=f"qT{i}", tag=f"qT{i}") for i in range(2)]
            kT_t = [qkv.tile([P, N], DT.bfloat16, name=f"kT{i}", tag=f"kT{i}") for i in range(2)]
            v_t = [qkv.tile([P, NH * 65], DT.bfloat16, name=f"v{j}", tag=f"v{j}") for j in range(JT)]
            aT_t = [qkv.tile([P, N], DT.bfloat16, name=f"aT{i}", tag=f"aT{i}") for i in range(2)]

            # ---- phase P: load inputs, project q/k/v ----
            with tc.tile_pool(name="inputs", bufs=1) as inp, \
                 tc.tile_pool(name="proj_ps", bufs=4, space="PSUM") as proj_ps, \
                 tc.tile_pool(name="v_ps", bufs=2, space="PSUM") as v_ps:
                xT_t = [inp.tile([P, N], DT.bfloat16, name=f"xT{c}", tag=f"xT{c}") for c in range(CT)]
                ctxT_t = [inp.tile([P, N], DT.bfloat16, name=f"cT{c}", tag=f"cT{c}") for c in range(CT)]
                wq_t = [inp.tile([P, E], DT.bfloat16, name=f"wq{c}", tag=f"wq{c}") for c in range(CT)]
                wk_t = [inp.tile([P, E], DT.bfloat16, name=f"wk{c}", tag=f"wk{c}") for c in range(CT)]
                wv_t = [inp.tile([P, E], DT.bfloat16, name=f"wv{c}", tag=f"wv{c}") for c in range(CT)]
                for c in range(CT):
                    r = slice(P * c, P * (c + 1))
                    nc.sync.dma_start(ctxT_t[c][:], ctxT_d[r, :])
                    nc.sync.dma_start(wk_t[c][:], wk_d[r, :])
                    nc.sync.dma_start(wv_t[c][:], wv_d[r, :])
                    nc.sync.dma_start(xT_t[c][:], xT_d[r, :])
                    nc.sync.dma_start(wq_t[c][:], wq_d[r, :])

                # kT[e, j] = Wk^T Ctx^T
                for et in range(2):
                    for jb in range(IB):
                        ps = proj_ps.tile([P, IBS], DT.float32, name="kps", tag="projps")
                        for c in range(CT):
                            nc.tensor.matmul(
                                ps[:], wk_t[c][:, P * et:P * (et + 1)],
                                ctxT_t[c][:, IBS * jb:IBS * (jb + 1)],
                                start=(c == 0), stop=(c == CT - 1))
                        nc.vector.tensor_copy(kT_t[et][:, IBS * jb:IBS * (jb + 1)], ps[:])
                # v[j, d] = Ctx Wv, interleaved with a ones column per head
                for j in range(JT):
                    ps = v_ps.tile([P, E], DT.float32, name="vps", tag="vps")
                    for c in range(CT):
                        nc.tensor.matmul(
                            ps[:], ctxT_t[c][:, P * j:P * (j + 1)], wv_t[c][:],
                            start=(c == 0), stop=(c == CT - 1))
                    v_re = v_t[j].rearrange("p (h x) -> p h x", h=NH)
                    nc.vector.tensor_copy(
                        v_re[:, :, 0:HD], ps.rearrange("p (h x) -> p h x", h=NH))
                    nc.vector.memset(v_re[:, :, HD:65], 1.0)
                # qT[e, i] = Wq^T X^T
                for et in range(2):
                    for ib in range(IB):
                        ps = proj_ps.tile([P, IBS], DT.float32, name="qps", tag="projps")
                        for c in range(CT):
                            nc.tensor.matmul(
                                ps[:], wq_t[c][:, P * et:P * (et + 1)],
                                xT_t[c][:, IBS * ib:IBS * (ib + 1)],
                                start=(c == 0), stop=(c == CT - 1))
                        nc.vector.tensor_copy(qT_t[et][:, IBS * ib:IBS * (ib + 1)], ps[:])

            # ---- MLP weights (loads overlap attention; reuse input space) ----
            mlpw = stack.enter_context(tc.tile_pool(name="mlpw", bufs=1))
            w1_t = [mlpw.tile([P, DIM], DT.bfloat16, name=f"w1_{c}", tag=f"w1_{c}") for c in range(CT)]
            w2_t = [mlpw.tile([P, DIM], DT.bfloat16, name=f"w2_{c}", tag=f"w2_{c}") for c in range(CT)]
            for c in range(CT):
                r = slice(P * c, P * (c + 1))
                nc.sync.dma_start(w1_t[c][:], w1_d[r, :])
                nc.sync.dma_start(w2_t[c][:], w2_d[r, :])

            # ---- phase A: attention, two heads (one qT/kT tile) at a time ----
            with tc.tile_pool(name="s_ps", bufs=2, space="PSUM") as s_ps_pool, \
                 tc.tile_pool(name="acc_ps", bufs=2, space="PSUM") as acc_pool, \
                 tc.tile_pool(name="p_sb", bufs=3) as p_pool, \
                 tc.tile_pool(name="nrm", bufs=4) as nrm_pool:
                for pr in range(2):
                    for ib in range(IB):
                        isl = slice(IBS * ib, IBS * (ib + 1))
                        accs = [acc_pool.tile([P, IBS], DT.float32, name=f"acc{hh}", tag=f"acc{hh}")
                                for hh in range(2)]
                        for j in range(JT):
                            sps = s_ps_pool.tile([P, 2 * IBS], DT.float32, name="sps", tag="sps")
                            for hh in range(2):
                                d = slice(HD * hh, HD * (hh + 1))
                                nc.tensor.matmul(
                                    sps[:, IBS * hh:IBS * (hh + 1)],
                                    kT_t[pr][d, P * j:P * (j + 1)], qT_t[pr][d, isl],
                                    start=True, stop=True)
                            pt = p_pool.tile([P, 2 * IBS], DT.bfloat16, name="pt", tag="pt")
                            nc.scalar.activation(pt[:], sps[:],
                                                 mybir.ActivationFunctionType.Exp,
                                                 scale=float(HD) ** -0.5)
                            for hh in range(2):
                                h = 2 * pr + hh
                                nc.tensor.matmul(
                                    accs[hh][0:65, :], v_t[j][:, 65 * h:65 * h + 65],
                                    pt[:, IBS * hh:IBS * (hh + 1)],
                                    start=(j == 0), stop=(j == JT - 1))
                        for hh in range(2):
                            rcp = nrm_pool.tile([1, IBS], DT.float32, name="rcp", tag="rcp")
                            nc.vector.reciprocal(rcp[:], accs[hh][64:65, :])
                            bc = nrm_pool.tile([HD, IBS], DT.float32, name="bc", tag="bc")
                            nc.gpsimd.partition_broadcast(bc[:], rcp[:])
                            nc.vector.tensor_tensor(
                                aT_t[pr][HD * hh:HD * (hh + 1), isl],
                                accs[hh][0:HD, :], bc[:], mybir.AluOpType.mult)

            # ---- phase C: AllToAll converts head-sharding -> row-sharding ----
            # 4-core-group AllToAll is unsupported (mesh needs >4 cores), so run
            # one 8-core AllToAll: every core sends its chunk for group-peer q to
            # BOTH absolute positions q and q+4; receivers read only the half
            # that corresponds to their own batch (cores 0-3 vs 4-7).
            a2a_in, a2a_in_free = tc.tile([8 * E, RQ], DT.bfloat16, space="DRAM",
                                          name="a2a_in")
            a2a_out, a2a_out_free = tc.tile([8 * E, RQ], DT.bfloat16, space="DRAM",
                                            addr_space="Shared", name="a2a_out")
            stack.callback(a2a_in_free)
            stack.callback(a2a_out_free)
            for q in range(4):
                for half in range(2):
                    src = aT_t[half][:, RQ * q:RQ * (q + 1)]
                    r0 = E * q + P * half
                    nc.sync.dma_start(a2a_in[r0:r0 + P, :], src)
                    nc.sync.dma_start(a2a_in[4 * E + r0:4 * E + r0 + P, :], src)
            nc.gpsimd.collective_compute(
                "AllToAll", mybir.AluOpType.bypass,
                replica_groups=[[0, 1, 2, 3, 4, 5, 6, 7]],
                ins=[a2a_in.opt()], outs=[a2a_out.opt()])

            mlp = stack.enter_context(tc.tile_pool(name="mlp", bufs=1))
            aTf_t = [mlp.tile([P, RQ], DT.bfloat16, name=f"aTf{c}", tag=f"aTf{c}") for c in range(CT)]
            hT_t = [mlp.tile([P, RQ], DT.bfloat16, name=f"hT{c}", tag=f"hT{c}") for c in range(CT)]
            pid = nc.sync.partition_id()
            with tc.If(pid < 4) as cmp:
                for c in range(CT):
                    nc.sync.dma_start(aTf_t[c][:], a2a_out[P * c:P * (c + 1), :])
            with cmp.Else():
                for c in range(CT):
                    nc.sync.dma_start(
                        aTf_t[c][:], a2a_out[4 * E + P * c:4 * E + P * (c + 1), :])

            # ---- phase M: Y^T = W1^T A^T (relu) ; Z = H W2 ; LayerNorm ----
            with tc.tile_pool(name="y_ps", bufs=2, space="PSUM") as y_ps_pool, \
                 tc.tile_pool(name="z_ps", bufs=2, space="PSUM") as z_ps_pool, \
                 tc.tile_pool(name="ln", bufs=2) as ln_pool:
                for ft in range(FT):
                    ps = y_ps_pool.tile([P, RQ], DT.float32, name="yps", tag="yps")
                    for et in range(CT):
                        nc.tensor.matmul(
                            ps[:], w1_t[et][:, P * ft:P * (ft + 1)], aTf_t[et][:],
                            start=(et == 0), stop=(et == CT - 1))
                    nc.vector.tensor_scalar_max(hT_t[ft][:], ps[:], 0.0)
                for it in range(RQ // P):
                    zps = z_ps_pool.tile([P, DIM], DT.float32, name="zps", tag="zps")
                    for gt in range(2):
                        for ft in range(FT):
                            nc.tensor.matmul(
                                zps[:, IBS * gt:IBS * (gt + 1)],
                                hT_t[ft][:, P * it:P * (it + 1)],
                                w2_t[ft][:, IBS * gt:IBS * (gt + 1)],
                                start=(ft == 0), stop=(ft == FT - 1))
                    stats = ln_pool.tile([P, 2, 6], DT.float32, name="stats", tag="stats")
                    for sg in range(2):
                        nc.vector.bn_stats(stats[:, sg, :], zps[:, IBS * sg:IBS * (sg + 1)])
                    mv = ln_pool.tile([P, 2], DT.float32, name="mv", tag="mv")
                    nc.vector.bn_aggr(mv[:], stats[:])
                    # mv[:,1] := 1/sqrt(var + eps)
                    nc.scalar.activation(mv[:, 1:2], mv[:, 1:2],
                                         mybir.ActivationFunctionType.Sqrt,
                                         bias=eps_t[:])
                    nc.vector.reciprocal(mv[:, 1:2], mv[:, 1:2])
                    zn = ln_pool.tile([P, DIM], DT.float32, name="zn", tag="zn")
                    nc.vector.tensor_scalar(
                        out=zn[:], in0=zps[:], scalar1=mv[:, 0:1], scalar2=mv[:, 1:2],
                        op0=mybir.AluOpType.subtract, op1=mybir.AluOpType.mult)
                    ot = ln_pool.tile([P, DIM], DT.float32, name="ot", tag="ot")
                    nc.vector.tensor_tensor(ot[:], zn[:], gamma_bc[:], mybir.AluOpType.mult)
                    nc.sync.dma_start(out_d[P * it:P * (it + 1), :], ot[:])

    nc.finalize()
    return nc


def _get_program():
    global _PROGRAM
    if _PROGRAM is None:
        _PROGRAM = build_program()
    return _PROGRAM


def prepare_in_maps(x, context, w_kv, w_q, w_out1, w_out2, gamma):
    x = np.asarray(x, np.float32)
    context = np.asarray(context, np.float32)
    w_kv = np.asarray(w_kv, np.float32)
    w_q = np.asarray(w_q, np.float32)
    w1 = np.ascontiguousarray(np.asarray(w_out1, np.float32).astype(BF16))
    w2 = np.ascontiguousarray(np.asarray(w_out2, np.float32).astype(BF16))
    gamma = np.asarray(gamma, np.float32).reshape(1, DIM)
    xT = [np.ascontiguousarray(x[b].T.astype(BF16)) for b in range(2)]
    ctxT = [np.ascontiguousarray(context[b].T.astype(BF16)) for b in range(2)]
    in_maps = []
    for c in range(8):
        b, g = divmod(c, 4)
        e0 = E * g
        in_maps.append({
            "xT": xT[b],
            "ctxT": ctxT[b],
            "wq": np.ascontiguousarray(w_q[:, e0:e0 + E].astype(BF16)),
            "wk": np.ascontiguousarray(w_kv[:, e0:e0 + E].astype(BF16)),
            "wv": np.ascontiguousarray(w_kv[:, DIM + e0:DIM + e0 + E].astype(BF16)),
            "w1": w1,
            "w2": w2,
            "gamma": gamma,
        })
    return in_maps


def assemble_output(per_core_outs):
    out = np.empty((2, N, DIM), np.float32)
    for c in range(8):
        b, g = divmod(c, 4)
        out[b, RQ * g:RQ * (g + 1), :] = per_core_outs[c]
    return out


def kernel(x, context, w_kv, w_q, w_out1, w_out2, gamma):
    global LAST_RUN
    in_maps = prepare_in_maps(x, context, w_kv, w_q, w_out1, w_out2, gamma)
    nc = _get_program()
    res = run_bass_kernel_spmd(nc, in_maps, list(range(8)))
    LAST_RUN = res
    return assemble_output([res.results[c]["out"] for c in range(8)])



# revision 8
# speedup vs baseline: 814.6251x; 814.6251x over previous
"""Trainium2 Bass kernel for nn_CrossAttention (b=2, n=m=2048, dim=1024, 16 heads x 64).

Sharding: 8 cores = (batch b in {0,1}) x (head-group g in {0..3}, 4 heads each).
Per core: project q/k/v for its 4 heads (feature-major layouts), attention with
softmax (no max subtraction -- logits are bounded ~|2.7|), row sums via a ones
column appended to V, then an AllToAll over the 4 cores of each batch converts
head-sharding to row-sharding for the output MLP (relu(A@W1)@W2) + LayerNorm.
"""

import sys

if "/opt/trn_rl_repo" not in sys.path:
    sys.path.insert(0, "/opt/trn_rl_repo")

from contextlib import ExitStack

import numpy as np
import ml_dtypes

import concourse.bacc as bacc
import concourse.tile as tile
from concourse import mybir, library_config
from concourse.bass_utils import run_bass_kernel_spmd

DT = mybir.dt
BF16 = ml_dtypes.bfloat16

P = 128          # partitions
N = 2048         # tokens per batch
DIM = 1024       # model dim
HD = 64          # head dim
NH = 4           # heads per core
E = NH * HD      # 256 features per core
CT = DIM // P    # 8 contraction tiles
JT = N // P      # 16 key tiles
IBS = 512        # i-block size
IB = N // IBS    # 4 i-blocks
RQ = 512         # output rows per core
FT = DIM // P    # 8 f-tiles in MLP

_PROGRAM = None
LAST_RUN = None  # BassKernelResults of the most recent kernel() call


def _emit_body(nc, tc, tensors, a2a_in, a2a_out, a2a_local=False):
    (xT_d, ctxT_d, wq_d, wk_d, wv_d, w1_d, w2_d, gamma_d, out_d) = tensors
    stack = ExitStack()
    if True:
        with stack:
            const = stack.enter_context(tc.tile_pool(name="const", bufs=1))
            gamma_sb = const.tile([1, DIM], DT.float32, name="gamma_sb", tag="gamma_sb")
            nc.sync.dma_start(gamma_sb[:], gamma_d[:])
            gamma_bc = const.tile([P, DIM], DT.float32, name="gamma_bc", tag="gamma_bc")
            nc.gpsimd.partition_broadcast(gamma_bc[:], gamma_sb[:])
            eps_t = const.tile([P, 1], DT.float32, name="eps_t", tag="eps_t")
            nc.vector.memset(eps_t[:], 1e-5)

            # ---- persistent activation tiles ----
            qkv = stack.enter_context(tc.tile_pool(name="qkv", bufs=1))
            qT_t = [qkv.tile([P, N], DT.bfloat16, name=f"qT{i}", tag=f"qT{i}") for i in range(2)]
            kT_t = [qkv.tile([P, N], DT.bfloat16, name=f"kT{i}", tag=f"kT{i}") for i in range(2)]
            v_t = [qkv.tile([P, NH * 65], DT.bfloat16, name=f"v{j}", tag=f"v{j}") for j in range(JT)]
            aT_t = [qkv.tile([P, N], DT.bfloat16, name=f"aT{i}", tag=f"aT{i}") for i in range(2)]

            # ---- phase P: load inputs, project q/k/v ----
            with tc.tile_pool(name="inputs", bufs=1) as inp, \
                 tc.tile_pool(name="proj_ps", bufs=4, space="PSUM") as proj_ps, \
                 tc.tile_pool(name="v_ps", bufs=2, space="PSUM") as v_ps:
                xT_t = [inp.tile([P, N], DT.bfloat16, name=f"xT{c}", tag=f"xT{c}") for c in range(CT)]
                ctxT_t = [inp.tile([P, N], DT.bfloat16, name=f"cT{c}", tag=f"cT{c}") for c in range(CT)]
                wq_t = [inp.tile([P, E], DT.bfloat16, name=f"wq{c}", tag=f"wq{c}") for c in range(CT)]
                wk_t = [inp.tile([P, E], DT.bfloat16, name=f"wk{c}", tag=f"wk{c}") for c in range(CT)]
                wv_t = [inp.tile([P, E], DT.bfloat16, name=f"wv{c}", tag=f"wv{c}") for c in range(CT)]
                for c in range(CT):
                    r = slice(P * c, P * (c + 1))
                    nc.sync.dma_start(ctxT_t[c][:], ctxT_d[r, :])
                    nc.sync.dma_start(wk_t[c][:], wk_d[r, :])
                    nc.sync.dma_start(wv_t[c][:], wv_d[r, :])
                    nc.sync.dma_start(xT_t[c][:], xT_d[r, :])
                    nc.sync.dma_start(wq_t[c][:], wq_d[r, :])

                # kT[e, j] = Wk^T Ctx^T
                for et in range(2):
                    for jb in range(IB):
                        ps = proj_ps.tile([P, IBS], DT.float32, name="kps", tag="projps")
                        for c in range(CT):
                            nc.tensor.matmul(
                                ps[:], wk_t[c][:, P * et:P * (et + 1)],
                                ctxT_t[c][:, IBS * jb:IBS * (jb + 1)],
                                start=(c == 0), stop=(c == CT - 1))
                        nc.vector.tensor_copy(kT_t[et][:, IBS * jb:IBS * (jb + 1)], ps[:])
                # v[j, d] = Ctx Wv, interleaved with a ones column per head
                for j in range(JT):
                    ps = v_ps.tile([P, E], DT.float32, name="vps", tag="vps")
                    for c in range(CT):
                        nc.tensor.matmul(
                            ps[:], ctxT_t[c][:, P * j:P * (j + 1)], wv_t[c][:],
                            start=(c == 0), stop=(c == CT - 1))
                    v_re = v_t[j].rearrange("p (h x) -> p h x", h=NH)
                    nc.vector.tensor_copy(
                        v_re[:, :, 0:HD], ps.rearrange("p (h x) -> p h x", h=NH))
                    nc.vector.memset(v_re[:, :, HD:65], 1.0)
                # qT[e, i] = Wq^T X^T
                for et in range(2):
                    for ib in range(IB):
                        ps = proj_ps.tile([P, IBS], DT.float32, name="qps", tag="projps")
                        for c in range(CT):
                            nc.tensor.matmul(
                                ps[:], wq_t[c][:, P * et:P * (et + 1)],
                                xT_t[c][:, IBS * ib:IBS * (ib + 1)],
                                start=(c == 0), stop=(c == CT - 1))
                        nc.vector.tensor_copy(qT_t[et][:, IBS * ib:IBS * (ib + 1)], ps[:])

            # ---- MLP weights (loads overlap attention; reuse input space) ----
            mlpw = stack.enter_context(tc.tile_pool(name="mlpw", bufs=1))
            w1_t = [mlpw.tile([P, DIM], DT.bfloat16, name=f"w1_{c}", tag=f"w1_{c}") for c in range(CT)]
            w2_t = [mlpw.tile([P, DIM], DT.bfloat16, name=f"w2_{c}", tag=f"w2_{c}") for c in range(CT)]
            for c in range(CT):
                r = slice(P * c, P * (c + 1))
                nc.sync.dma_start(w1_t[c][:], w1_d[r, :])
                nc.sync.dma_start(w2_t[c][:], w2_d[r, :])

            # ---- phase A: attention, two heads (one qT/kT tile) at a time ----
            with tc.tile_pool(name="s_ps", bufs=2, space="PSUM") as s_ps_pool, \
                 tc.tile_pool(name="acc_ps", bufs=2, space="PSUM") as acc_pool, \
                 tc.tile_pool(name="p_sb", bufs=3) as p_pool, \
                 tc.tile_pool(name="nrm", bufs=4) as nrm_pool:
                for pr in range(2):
                    for ib in range(IB):
                        isl = slice(IBS * ib, IBS * (ib + 1))
                        accs = [acc_pool.tile([P, IBS], DT.float32, name=f"acc{hh}", tag=f"acc{hh}")
                                for hh in range(2)]
                        for j in range(JT):
                            sps = s_ps_pool.tile([P, 2 * IBS], DT.float32, name="sps", tag="sps")
                            for hh in range(2):
                                d = slice(HD * hh, HD * (hh + 1))
                                nc.tensor.matmul(
                                    sps[:, IBS * hh:IBS * (hh + 1)],
                                    kT_t[pr][d, P * j:P * (j + 1)], qT_t[pr][d, isl],
                                    start=True, stop=True)
                            pt = p_pool.tile([P, 2 * IBS], DT.bfloat16, name="pt", tag="pt")
                            nc.scalar.activation(pt[:], sps[:],
                                                 mybir.ActivationFunctionType.Exp,
                                                 scale=float(HD) ** -0.5)
                            for hh in range(2):
                                h = 2 * pr + hh
                                nc.tensor.matmul(
                                    accs[hh][0:65, :], v_t[j][:, 65 * h:65 * h + 65],
                                    pt[:, IBS * hh:IBS * (hh + 1)],
                                    start=(j == 0), stop=(j == JT - 1))
                        for hh in range(2):
                            rcp = nrm_pool.tile([1, IBS], DT.float32, name="rcp", tag="rcp")
                            nc.vector.reciprocal(rcp[:], accs[hh][64:65, :])
                            bc = nrm_pool.tile([HD, IBS], DT.float32, name="bc", tag="bc")
                            nc.gpsimd.partition_broadcast(bc[:], rcp[:])
                            nc.vector.tensor_tensor(
                                aT_t[pr][HD * hh:HD * (hh + 1), isl],
                                accs[hh][0:HD, :], bc[:], mybir.AluOpType.mult)

            # ---- phase C: AllToAll converts head-sharding -> row-sharding ----
            # 4-core-group AllToAll is unsupported (mesh needs >4 cores), so run
            # one 8-core AllToAll: every core sends its chunk for group-peer q to
            # BOTH absolute positions q and q+4; receivers read only the half
            # that corresponds to their own batch (cores 0-3 vs 4-7).
            for q in range(4):
                for half in range(2):
                    src = aT_t[half][:, RQ * q:RQ * (q + 1)]
                    r0 = E * q + P * half
                    nc.sync.dma_start(a2a_in[r0:r0 + P, :], src)
                    nc.sync.dma_start(a2a_in[4 * E + r0:4 * E + r0 + P, :], src)
            if a2a_local:
                # TimelineSim proxy: model the collective as a local DRAM->DRAM
                # copy of the same footprint.
                nc.sync.dma_start(a2a_out[:, :], a2a_in[:, :])
            else:
                nc.gpsimd.collective_compute(
                    "AllToAll", mybir.AluOpType.bypass,
                    replica_groups=[[0, 1, 2, 3, 4, 5, 6, 7]],
                    ins=[a2a_in.opt()], outs=[a2a_out.opt()])

            mlp = stack.enter_context(tc.tile_pool(name="mlp", bufs=1))
            aTf_t = [mlp.tile([P, RQ], DT.bfloat16, name=f"aTf{c}", tag=f"aTf{c}") for c in range(CT)]
            hT_t = [mlp.tile([P, RQ], DT.bfloat16, name=f"hT{c}", tag=f"hT{c}") for c in range(CT)]
            pid = nc.sync.partition_id()
            with tc.If(pid < 4) as cmp:
                for c in range(CT):
                    nc.sync.dma_start(aTf_t[c][:], a2a_out[P * c:P * (c + 1), :])
            with cmp.Else():
                for c in range(CT):
                    nc.sync.dma_start(
                        aTf_t[c][:], a2a_out[4 * E + P * c:4 * E + P * (c + 1), :])

            # ---- phase M: Y^T = W1^T A^T (relu) ; Z = H W2 ; LayerNorm ----
            with tc.tile_pool(name="y_ps", bufs=2, space="PSUM") as y_ps_pool, \
                 tc.tile_pool(name="z_ps", bufs=2, space="PSUM") as z_ps_pool, \
                 tc.tile_pool(name="ln", bufs=2) as ln_pool:
                for ft in range(FT):
                    ps = y_ps_pool.tile([P, RQ], DT.float32, name="yps", tag="yps")
                    for et in range(CT):
                        nc.tensor.matmul(
                            ps[:], w1_t[et][:, P * ft:P * (ft + 1)], aTf_t[et][:],
                            start=(et == 0), stop=(et == CT - 1))
                    nc.vector.tensor_scalar_max(hT_t[ft][:], ps[:], 0.0)
                for it in range(RQ // P):
                    zps = z_ps_pool.tile([P, DIM], DT.float32, name="zps", tag="zps")
                    for gt in range(2):
                        for ft in range(FT):
                            nc.tensor.matmul(
                                zps[:, IBS * gt:IBS * (gt + 1)],
                                hT_t[ft][:, P * it:P * (it + 1)],
                                w2_t[ft][:, IBS * gt:IBS * (gt + 1)],
                                start=(ft == 0), stop=(ft == FT - 1))
                    stats = ln_pool.tile([P, 2, 6], DT.float32, name="stats", tag="stats")
                    for sg in range(2):
                        nc.vector.bn_stats(stats[:, sg, :], zps[:, IBS * sg:IBS * (sg + 1)])
                    mv = ln_pool.tile([P, 2], DT.float32, name="mv", tag="mv")
                    nc.vector.bn_aggr(mv[:], stats[:])
                    # mv[:,1] := 1/sqrt(var + eps)
                    nc.scalar.activation(mv[:, 1:2], mv[:, 1:2],
                                         mybir.ActivationFunctionType.Sqrt,
                                         bias=eps_t[:])
                    nc.vector.reciprocal(mv[:, 1:2], mv[:, 1:2])
                    zn = ln_pool.tile([P, DIM], DT.float32, name="zn", tag="zn")
                    nc.vector.tensor_scalar(
                        out=zn[:], in0=zps[:], scalar1=mv[:, 0:1], scalar2=mv[:, 1:2],
                        op0=mybir.AluOpType.subtract, op1=mybir.AluOpType.mult)
                    ot = ln_pool.tile([P, DIM], DT.float32, name="ot", tag="ot")
                    nc.vector.tensor_tensor(ot[:], zn[:], gamma_bc[:], mybir.AluOpType.mult)
                    nc.sync.dma_start(out_d[P * it:P * (it + 1), :], ot[:])


def _build(k_rep, a2a_local=False):
    nc = bacc.Bacc(None, num_devices=8)

    xT_d = nc.dram_tensor("xT", [DIM, N], DT.bfloat16, kind="ExternalInput")
    ctxT_d = nc.dram_tensor("ctxT", [DIM, N], DT.bfloat16, kind="ExternalInput")
    wq_d = nc.dram_tensor("wq", [DIM, E], DT.bfloat16, kind="ExternalInput")
    wk_d = nc.dram_tensor("wk", [DIM, E], DT.bfloat16, kind="ExternalInput")
    wv_d = nc.dram_tensor("wv", [DIM, E], DT.bfloat16, kind="ExternalInput")
    w1_d = nc.dram_tensor("w1", [DIM, DIM], DT.bfloat16, kind="ExternalInput")
    w2_d = nc.dram_tensor("w2", [DIM, DIM], DT.bfloat16, kind="ExternalInput")
    gamma_d = nc.dram_tensor("gamma", [1, DIM], DT.float32, kind="ExternalInput")
    out_d = nc.dram_tensor("out", [RQ, DIM], DT.float32, kind="ExternalOutput")
    tensors = (xT_d, ctxT_d, wq_d, wk_d, wv_d, w1_d, w2_d, gamma_d, out_d)

    with tile.TileContext(nc) as tc:
        nc.gpsimd.load_library(library_config.attnmlp)
        a2a_in, a2a_in_free = tc.tile([8 * E, RQ], DT.bfloat16, space="DRAM",
                                      name="a2a_in")
        a2a_out, a2a_out_free = tc.tile([8 * E, RQ], DT.bfloat16, space="DRAM",
                                        addr_space="Shared", name="a2a_out")
        for _ in range(k_rep):
            _emit_body(nc, tc, tensors, a2a_in, a2a_out, a2a_local=a2a_local)
        a2a_in_free()
        a2a_out_free()

    nc.finalize()
    return nc


def build_program():
    return _build(1)


def build_program_k(k_rep):
    return _build(k_rep)


def _get_program():
    global _PROGRAM
    if _PROGRAM is None:
        _PROGRAM = build_program()
    return _PROGRAM


def prepare_in_maps(x, context, w_kv, w_q, w_out1, w_out2, gamma):
    x = np.asarray(x, np.float32)
    context = np.asarray(context, np.float32)
    w_kv = np.asarray(w_kv, np.float32)
    w_q = np.asarray(w_q, np.float32)
    w1 = np.ascontiguousarray(np.asarray(w_out1, np.float32).astype(BF16))
    w2 = np.ascontiguousarray(np.asarray(w_out2, np.float32).astype(BF16))
    gamma = np.asarray(gamma, np.float32).reshape(1, DIM)
    xT = [np.ascontiguousarray(x[b].T.astype(BF16)) for b in range(2)]
    ctxT = [np.ascontiguousarray(context[b].T.astype(BF16)) for b in range(2)]
    in_maps = []
    for c in range(8):
        b, g = divmod(c, 4)
        e0 = E * g
        in_maps.append({
            "xT": xT[b],
            "ctxT": ctxT[b],
            "wq": np.ascontiguousarray(w_q[:, e0:e0 + E].astype(BF16)),
            "wk": np.ascontiguousarray(w_kv[:, e0:e0 + E].astype(BF16)),
            "wv": np.ascontiguousarray(w_kv[:, DIM + e0:DIM + e0 + E].astype(BF16)),
            "w1": w1,
            "w2": w2,
            "gamma": gamma,
        })
    return in_maps


def assemble_output(per_core_outs):
    out = np.empty((2, N, DIM), np.float32)
    for c in range(8):
        b, g = divmod(c, 4)
        out[b, RQ * g:RQ * (g + 1), :] = per_core_outs[c]
    return out


def kernel(x, context, w_kv, w_q, w_out1, w_out2, gamma):
    global LAST_RUN
    in_maps = prepare_in_maps(x, context, w_kv, w_q, w_out1, w_out2, gamma)
    nc = _get_program()
    res = run_bass_kernel_spmd(nc, in_maps, list(range(8)))
    LAST_RUN = res
    return assemble_output([res.results[c]["out"] for c in range(8)])

